# revision 7
# baseline (speedup 1.0000x reference)
"""Trainium2 Bass kernel for nn_Disp_61125974557155 (V1: all-bf16, R-stationary stats).

Computes: trilinear upsample of a cost volume [B,1,48,64,128] ->
[B,193,256,512] (align_corners=False, edge-replicated), softmin over
disparity, disparity regression -> [B,256,512].

Design (per core; 8 cores = 2 batches x 4 H'-quarters):
  - Host: edge-pad x (replicate), slice the core's H-halo shard, stack a
    copy shifted by one h-row on partitions 50..99, cast bf16 (sharding /
    layout only).
  - DVE: W-axis 4x lerp at low resolution -> xsw [100, 19, 4, 128] bf16.
  - PE: D-expansion (48->193 banded lerp matrix, bf16) with the H-axis 4x
    lerp FOLDED into the matmul via the dup-shifted operand halves:
    vol = A2r^T @ xsw_row -> PSUM [d'-chunk, 512] tiles.
  - ACT: e = exp(-vol) (PSUM -> SBUF, bf16), FD-1024 tiles.
  - PE: stats with rmat STATIONARY ([dn, 2] = {1, d}) and e MOVING:
    out [2, 512] per (t, r) accumulated over the two d'-chunks. This
    streams e through the array at 128 elem/cycle @ 2.4 GHz instead of
    paying a 128-col LDWEIGHTS (1.2 GHz) per 2 streamed columns like the
    flipped layout does.
  - DVE: copy stat rows [2, 2048] PSUM->SBUF per t; DMA scatters them
    SBUF->SBUF into pixel-major [64, 512] S0/S1; DVE recip+mul finalize;
    DMA out.
"""

import numpy as np
from contextlib import ExitStack

import concourse.bass as bass
import concourse.bacc as bacc
import concourse.tile as tile
from concourse import mybir
from concourse.bass_utils import run_bass_kernel_spmd

F32 = mybir.dt.float32
BF16 = mybir.dt.bfloat16

MAXDISP = 192
DP = MAXDISP + 1      # 193 disparities
KD = 48               # low-res D
KP = KD + 2           # padded k' (edge-replicated)
NCORES = 8
WH = (0.625, 0.875, 0.125, 0.375)   # H lerp fracs per r = h' % 4
CHUNKS = ((0, 128), (128, 65))      # d' chunk (offset, size)
NROW = 19                            # h-rows in dup-packed shard
ROW_GROUPS = ((0, 2), (2, 2), (4, 4), (8, 4), (12, 4), (16, 1))


def _build_ad() -> np.ndarray:
    """A_D [193, 50]: D-axis linear upsample matrix on padded k' = k+1."""
    ad = np.zeros((DP, KP), dtype=np.float64)
    for dp in range(DP):
        i = (dp + 0.5) * KD / DP - 0.5
        fl = int(np.floor(i))
        fr = i - fl
        ad[dp, fl + 1] += 1.0 - fr
        ad[dp, fl + 2] += fr
    return ad


def _build_consts():
    ad = _build_ad()                      # [193, 50]
    amat = np.zeros((2 * KP, 4, DP), dtype=np.float64)
    for r in range(4):
        amat[:KP, r, :] = (1.0 - WH[r]) * ad.T
        amat[KP:, r, :] = WH[r] * ad.T
    rmat = np.zeros((128, 4), dtype=np.float64)
    rmat[:, 0] = 1.0
    rmat[:, 1] = np.arange(128)
    rmat[: DP - 128, 2] = 1.0
    rmat[: DP - 128, 3] = np.arange(128, DP)
    bf = mybir.dt.np(BF16)
    return (
        np.ascontiguousarray(amat.reshape(2 * KP, 4 * DP)).astype(bf),
        rmat.astype(np.float32).astype(bf),
    )


def _build_nc() -> bass.Bass:
    nc = bacc.Bacc()
    xsd = nc.declare_dram_parameter("xsd", [2 * KP, NROW * 130], BF16, isOutput=False)
    amat = nc.declare_dram_parameter("amat", [2 * KP, 4 * DP], BF16, isOutput=False)
    rmat = nc.declare_dram_parameter("rmat", [128, 4], BF16, isOutput=False)
    outp = nc.declare_dram_parameter("out", [64, 512], F32, isOutput=True)

    xsd_v = xsd.rearrange("p (h w) -> p h w", h=NROW)
    amat_v = amat.rearrange("p (r d) -> p r d", r=4)

    mult = mybir.AluOpType.mult
    add = mybir.AluOpType.add
    exp_fn = mybir.ActivationFunctionType.Exp

    with ExitStack() as ctx:
        tc = ctx.enter_context(tile.TileContext(nc))
        singles = ctx.enter_context(tc.tile_pool(name="singles", bufs=1))
        tmp_pool = ctx.enter_context(tc.tile_pool(name="tmp", bufs=2))
        epool = ctx.enter_context(tc.tile_pool(name="epool", bufs=8))
        dpool = ctx.enter_context(tc.tile_pool(name="dpool", bufs=2))
        fin = ctx.enter_context(tc.tile_pool(name="fin", bufs=1))
        pvol = ctx.enter_context(tc.tile_pool(name="pvol", bufs=2, space="PSUM"))
        pstat = ctx.enter_context(tc.tile_pool(name="pstat", bufs=1, space="PSUM"))

        # ---- input loads: xsd first (gates the lerp chain) on the sync
        # HWDGE queue; constants go through gpsimd SWDGE in parallel ----
        s_xsd = []
        for g, (g0, gn) in enumerate(ROW_GROUPS):
            t_x = singles.tile([2 * KP, gn, 130], BF16, tag=f"xsd{g}")
            nc.sync.dma_start(out=t_x, in_=xsd_v[:, g0 : g0 + gn, :])
            s_xsd.append(t_x)
        s_am = {}
        for ci, (d0, dn) in enumerate(CHUNKS):
            for r in range(4):
                t_a = singles.tile([2 * KP, dn], BF16, tag=f"am{ci}{r}")
                nc.gpsimd.dma_start(out=t_a, in_=amat_v[:, r, d0 : d0 + dn])
                s_am[(ci, r)] = t_a
        s_rm = singles.tile([128, 4], BF16, tag="rm")
        nc.gpsimd.dma_start(out=s_rm, in_=rmat[:, :])

        # ---- W-axis 4x lerp at low res, rw-major planes (bf16, DVE 2x) ----
        # xsw[p, h, rw, s] = lerp; shared difference d[s] = xs[s] - xs[s+1]:
        #   rw0 = xs[s+1] + 0.375*d[s]    rw1 = xs[s+1] + 0.125*d[s]
        #   rw2 = xs[s+2] + 0.875*d[s+1]  rw3 = xs[s+2] + 0.625*d[s+1]
        s_xsw = []
        for g, (g0, gn) in enumerate(ROW_GROUPS):
            t_w = singles.tile([2 * KP, gn, 4, 128], BF16, tag=f"xsw{g}")
            t_d = tmp_pool.tile([2 * KP, gn, 129], BF16, tag="wld")
            nc.vector.tensor_sub(
                t_d, s_xsd[g][:, :, 0:129], s_xsd[g][:, :, 1:130]
            )
            for rw, (coef, dc, hc) in enumerate(
                ((0.375, 0, 1), (0.125, 0, 1), (0.875, 1, 2), (0.625, 1, 2))
            ):
                nc.vector.scalar_tensor_tensor(
                    out=t_w[:, :, rw, :],
                    in0=t_d[:, :, dc : dc + 128],
                    scalar=coef,
                    in1=s_xsd[g][:, :, hc : hc + 128],
                    op0=mult,
                    op1=add,
                )
            s_xsw.append(t_w)

        def xsw_row(l: int) -> bass.AP:
            for g, (g0, gn) in enumerate(ROW_GROUPS):
                if g0 <= l < g0 + gn:
                    return s_xsw[g][:, l - g0, :, :]
            raise IndexError(l)

        # ---- S0/S1 pixel-major accumulators (filled by DMA scatter) ----
        s0mat = fin.tile([64, 512], F32, tag="s0mat")
        s1mat = fin.tile([64, 512], F32, tag="s1mat")

        # ---- main loop over coarse h-rows t; r = h' % 4, j = 4t + r ----
        # Stats for 4 consecutive t live in one pstat tile (bank = r,
        # partition base = 32*(t%4) via tile_position), so the PSUM->SBUF
        # drain runs once per 4 t. Stat matmuls are emitted one t behind
        # the vol/exp chain so the PE never waits on ACT output.
        from collections import deque

        pending = deque()

        def emit_stats(ps, et, ci, tau, rp, is_block_end, t0):
            for u in range(2):
                r = 2 * rp + u
                nc.tensor.matmul(
                    ps[32 * tau : 32 * tau + 2, r, :],
                    s_rm[0:CHUNKS[ci][1], 2 * ci : 2 * ci + 2],
                    et[0:CHUNKS[ci][1], u, :],
                    start=(ci == 0),
                    stop=(ci == 1),
                    skip_group_check=True,
                    tile_position=(0, 32 * tau),
                )
            if is_block_end:
                sd = dpool.tile([128, 4, 512], F32, tag="sd")
                nc.vector.tensor_copy(sd[0:98, :, :], ps[0:98, :, :])
                for tau2 in range(4):
                    j0 = 4 * (t0 + tau2)
                    nc.sync.dma_start(
                        out=s0mat[j0 : j0 + 4, :],
                        in_=sd[32 * tau2 : 32 * tau2 + 1, :, :],
                    )
                    nc.sync.dma_start(
                        out=s1mat[j0 : j0 + 4, :],
                        in_=sd[32 * tau2 + 1 : 32 * tau2 + 2, :, :],
                    )

        ps = None
        for t in range(16):
            if t % 4 == 0:
                ps = pstat.tile([128, 4, 512], F32, tag="ps")
            tau = t % 4
            for ci, (d0, dn) in enumerate(CHUNKS):
                for rp in range(2):
                    pv = pvol.tile([128, 2, 512], F32, tag="pv")
                    et = epool.tile([128, 2, 512], BF16, tag="e")
                    for u in range(2):
                        r = 2 * rp + u
                        l = t if r < 2 else t + 1
                        rhs = xsw_row(l).rearrange("p q s -> p (q s)")
                        nc.tensor.matmul(
                            pv[0:dn, u, :],
                            s_am[(ci, r)][:, :],
                            rhs,
                            start=True,
                            stop=True,
                        )
                    nc.scalar.activation(
                        et[0:dn, :, :], pv[0:dn, :, :], exp_fn, scale=-1.0
                    )
                    pending.append(
                        (ps, et, ci, tau, rp,
                         ci == 1 and rp == 1 and tau == 3, t - tau)
                    )
                    if len(pending) > 4:
                        emit_stats(*pending.popleft())
        while pending:
            emit_stats(*pending.popleft())

        # ---- finalize: disp = S1 * recip(S0), partition-parallel ----
        # stat columns are in (rw, s) order (pixel w' = 4s + rw); un-permute
        # to w'-order during the multiply via a strided output AP.
        rec = fin.tile([64, 512], F32, tag="rec")
        om = fin.tile([64, 512], F32, tag="om")
        nc.vector.reciprocal(rec, s0mat)
        nc.vector.tensor_mul(
            om.rearrange("j (s q) -> j q s", q=4),
            s1mat.rearrange("j (q s) -> j q s", q=4),
            rec.rearrange("j (q s) -> j q s", q=4),
        )
        nc.sync.dma_start(out=outp[:, :], in_=om)

    nc.compile()
    return nc


_CACHE: dict = {}


def _shard_inputs(x: np.ndarray):
    """Edge-pad and slice per-core shards (layout + dtype cast only)."""
    xpad = np.pad(x[:, 0], ((0, 0), (1, 1), (1, 3), (1, 1)), mode="edge")
    amat, rmat = _build_consts()
    bf = mybir.dt.np(BF16)
    in_maps = []
    for c in range(NCORES):
        b, q = divmod(c, 4)
        xs = xpad[b][:, 16 * q : 16 * q + 20, :]          # [50, 20, 130]
        xsd = np.concatenate([xs[:, 0:19, :], xs[:, 1:20, :]], axis=0)
        xsd = np.ascontiguousarray(xsd.reshape(2 * KP, NROW * 130)).astype(bf)
        in_maps.append({"xsd": xsd, "amat": amat, "rmat": rmat})
    return in_maps


def kernel(x: np.ndarray, _trace: bool = False, _tmpdir=None):
    x = np.asarray(x, dtype=np.float32)
    assert x.shape == (2, 1, 48, 64, 128), x.shape
    if "nc" not in _CACHE:
        _CACHE["nc"] = _build_nc()
    nc = _CACHE["nc"]
    in_maps = _shard_inputs(x)
    res = run_bass_kernel_spmd(
        nc, in_maps, list(range(NCORES)), trace=_trace, tmpdir=_tmpdir
    )
    out = np.zeros((2, 256, 512), dtype=np.float32)
    for c in range(NCORES):
        b, q = divmod(c, 4)
        out[b, 64 * q : 64 * (q + 1), :] = res.results[c]["out"]
    if _trace:
        return out, res
    return out


# revision 9
# speedup vs baseline: 1.4094x; 1.4094x over previous
"""Trainium2 Bass kernel for nn_Disp_61125974557155 (V2: segment-geometric exp).

Computes: trilinear upsample of a cost volume [B,1,48,64,128] ->
[B,193,256,512] (align_corners=False), softmin over disparity,
disparity regression -> [B,256,512].

Key idea: along d', the upsampled volume is piecewise-linear over 49
low-res segments (~4 d' each), so exp(-vol) is piecewise GEOMETRIC:
within segment s, e_j = A_s * q_s^j with A_s = exp(-(u_s + f0*Delta)),
q_s = exp(-c*Delta), c = 48/193. Instead of 193 exp rows per pixel the
scalar engine computes only ~194 rows per (t, r-pair) (A and q for two
H-phases), and the idle vector engine reconstructs the remaining rows
with chained bf16 multiplies (2x DVE mode). Edge segments (replicated
pads, Delta=0, 2 d's each) are folded into the j=0 stat weights; the
single 5-element segment gets its 5th row directly from the alpha
matmul (alpha + 4w column).

Per core (8 = 2 batches x 4 H'-quarters), t-major over 16 coarse rows:
  PE:  alpha/w matmul [100->100/94, 512] per (t, rp) (H-lerp folded),
       then 8 masked stat matmuls (M=4: S0/S1 x r-even/r-odd) per t,
       accumulated over layers j=0..3 in PSUM slots at partition 32*(t%4).
  ACT: one exp over [100, 4, 512] PSUM -> SBUF bf16 per t.
  DVE: W-lerp (bf16), 3 reconstruction TT muls per t, stat drains every
       4 t, partition-parallel finalize.
  DMA: stat scatter to pixel-major S0/S1, final divide, output.
"""

import numpy as np
from contextlib import ExitStack

import concourse.bass as bass
import concourse.bacc as bacc
import concourse.tile as tile
from concourse import mybir
from concourse.bass_utils import run_bass_kernel_spmd

F32 = mybir.dt.float32
BF16 = mybir.dt.bfloat16

MAXDISP = 192
DP = MAXDISP + 1
KD = 48
KP = KD + 2
NCORES = 8
C2 = KD / DP
WH = (0.625, 0.875, 0.125, 0.375)
NROW = 17
ROW_GROUPS = ((0, 1), (1, 1), (2, 2), (4, 4), (8, 4), (12, 4), (16, 1))


def _segments():
    segs = {}
    for dd in range(DP):
        i = (dd + 0.5) * C2 - 0.5
        fl = int(np.floor(i))
        segs.setdefault(fl, []).append((dd, i - fl))
    int_fls = [fl for fl in sorted(segs) if 0 <= fl <= 46]
    edge_fls = [-1, 47]
    five = [fl for fl in int_fls if len(segs[fl]) == 5]
    assert len(int_fls) == 47 and len(five) == 1
    return segs, int_fls, edge_fls, five[0]


def _build_consts():
    segs, int_fls, edge_fls, five_fl = _segments()

    def hrow(r, k, wt):
        v = np.zeros(100)
        v[k] = (1 - WH[r]) * wt
        v[50 + k] = WH[r] * wt
        return v

    awA = np.zeros((2, 100, 100))
    awW = np.zeros((2, 100, 94))
    for rp in range(2):
        for ui, r in enumerate((2 * rp, 2 * rp + 1)):
            for s, fl in enumerate(int_fls):
                klo = fl + 1
                f0 = segs[fl][0][1]
                awA[rp][:, 47 * ui + s] = hrow(r, klo, 1 - f0) + hrow(r, klo + 1, f0)
                awW[rp][:, 47 * ui + s] = hrow(r, klo + 1, C2) + hrow(r, klo, -C2)
            for ei, fl in enumerate(edge_fls):
                klo = fl + 1
                f0 = segs[fl][0][1]
                awA[rp][:, 94 + 2 * ui + ei] = (
                    hrow(r, klo, 1 - f0) + hrow(r, klo + 1, f0)
                )
            klo = five_fl + 1
            f0 = segs[five_fl][0][1]
            a_col = hrow(r, klo, 1 - f0) + hrow(r, klo + 1, f0)
            w_col = hrow(r, klo + 1, C2) + hrow(r, klo, -C2)
            awA[rp][:, 98 + ui] = a_col + 4 * w_col

    # stat cols ordered (S0_r0, S0_r1, S1_r0, S1_r1) for contiguous scatters
    rmA = np.zeros((100, 4))
    rmE = np.zeros((3, 100, 4))
    for ui in range(2):
        for s, fl in enumerate(int_fls):
            ds = [t[0] for t in segs[fl]]
            rmA[47 * ui + s, ui] = 1
            rmA[47 * ui + s, 2 + ui] = ds[0]
            for j in (1, 2, 3):
                rmE[j - 1][47 * ui + s, ui] = 1
                rmE[j - 1][47 * ui + s, 2 + ui] = ds[j]
        for ei, fl in enumerate(edge_fls):
            ds = [t[0] for t in segs[fl]]
            rmA[94 + 2 * ui + ei, ui] = 2
            rmA[94 + 2 * ui + ei, 2 + ui] = ds[0] + ds[1]
        ds = [t[0] for t in segs[five_fl]]
        rmA[98 + ui, ui] = 1
        rmA[98 + ui, 2 + ui] = ds[4]

    bf = mybir.dt.np(BF16)
    aw = np.concatenate(
        [awA[0], awW[0], awA[1], awW[1]], axis=1
    )  # [100, 388]
    rm = np.concatenate([rmA] + [rmE[j] for j in range(3)], axis=1)  # [100, 16]
    return aw.astype(bf), rm.astype(bf)


def _build_nc() -> bass.Bass:
    nc = bacc.Bacc()
    # xsdA/xsdB: host-duplicated, B shifted by one w-col, so every lerp
    # slice starts at an even element offset (keeps DVE 2x eligibility).
    xsdA = nc.declare_dram_parameter("xsdA", [100, NROW * 130], BF16, isOutput=False)
    xsdB = nc.declare_dram_parameter("xsdB", [100, NROW * 130], BF16, isOutput=False)
    aw = nc.declare_dram_parameter("aw", [100, 388], BF16, isOutput=False)
    rm = nc.declare_dram_parameter("rm", [100, 16], BF16, isOutput=False)
    outp = nc.declare_dram_parameter("out", [64, 512], F32, isOutput=True)

    xa_v = xsdA.rearrange("p (h w) -> p h w", h=NROW)
    xb_v = xsdB.rearrange("p (h w) -> p h w", h=NROW)
    aw_v = aw  # cols: rp0A(100) rp0W(94) rp1A(100) rp1W(94)

    mult = mybir.AluOpType.mult
    add = mybir.AluOpType.add
    exp_fn = mybir.ActivationFunctionType.Exp

    with ExitStack() as ctx:
        tc = ctx.enter_context(tile.TileContext(nc))
        singles = ctx.enter_context(tc.tile_pool(name="singles", bufs=1))
        tmp_pool = ctx.enter_context(tc.tile_pool(name="tmp", bufs=2))
        epool = ctx.enter_context(tc.tile_pool(name="epool", bufs=4))
        erp = ctx.enter_context(tc.tile_pool(name="erp", bufs=3))
        dpool = ctx.enter_context(tc.tile_pool(name="dpool", bufs=2))
        fin = ctx.enter_context(tc.tile_pool(name="fin", bufs=1))
        paw = ctx.enter_context(tc.tile_pool(name="paw", bufs=1, space="PSUM"))
        pstat = ctx.enter_context(tc.tile_pool(name="pstat", bufs=2, space="PSUM"))

        # ---- loads ----
        s_xa, s_xb = [], []
        for g, (g0, gn) in enumerate(ROW_GROUPS):
            t_xa = singles.tile([100, gn, 130], BF16, tag=f"xa{g}")
            nc.sync.dma_start(out=t_xa, in_=xa_v[:, g0 : g0 + gn, :])
            t_xb = singles.tile([100, gn, 130], BF16, tag=f"xb{g}")
            nc.scalar.dma_start(out=t_xb, in_=xb_v[:, g0 : g0 + gn, :])
            s_xa.append(t_xa)
            s_xb.append(t_xb)
        s_aw = {}
        off = 0
        for rp in range(2):
            t_a = singles.tile([100, 100], BF16, tag=f"awa{rp}")
            nc.gpsimd.dma_start(out=t_a, in_=aw_v[:, off : off + 100])
            off += 100
            t_w = singles.tile([100, 94], BF16, tag=f"aww{rp}")
            nc.gpsimd.dma_start(out=t_w, in_=aw_v[:, off : off + 94])
            off += 94
            s_aw[(rp, 0)] = t_a
            s_aw[(rp, 1)] = t_w
        s_rm = singles.tile([100, 4, 4], BF16, tag="rm")
        nc.gpsimd.dma_start(out=s_rm, in_=rm.rearrange("p (j c) -> p j c", j=4))

        # ---- W-lerp (bf16): rw0/1 = xsB[s] + c*(xsA[s]-xsB[s]);
        #      rw2/3 = xsA[s+2] + c*(xsB[s]-xsA[s+2])  (all even offsets) ----
        s_xsw = []
        for g, (g0, gn) in enumerate(ROW_GROUPS):
            t_w = singles.tile([100, gn, 4, 128], BF16, tag=f"xsw{g}")
            d0 = tmp_pool.tile([100, gn, 128], BF16, tag="d0")
            d1 = tmp_pool.tile([100, gn, 128], BF16, tag="d1")
            nc.vector.tensor_sub(d0, s_xa[g][:, :, 0:128], s_xb[g][:, :, 0:128])
            nc.vector.tensor_sub(d1, s_xb[g][:, :, 0:128], s_xa[g][:, :, 2:130])
            for rw, (coef, dt_, base) in enumerate(
                (
                    (0.375, 0, 0),
                    (0.125, 0, 0),
                    (0.875, 1, 2),
                    (0.625, 1, 2),
                )
            ):
                src_d = d0 if dt_ == 0 else d1
                src_x = (
                    s_xb[g][:, :, 0:128] if dt_ == 0 else s_xa[g][:, :, 2:130]
                )
                nc.vector.scalar_tensor_tensor(
                    out=t_w[:, :, rw, :],
                    in0=src_d,
                    scalar=coef,
                    in1=src_x,
                    op0=mult,
                    op1=add,
                )
            s_xsw.append(t_w)

        def xsw_row(l: int) -> bass.AP:
            for g, (g0, gn) in enumerate(ROW_GROUPS):
                if g0 <= l < g0 + gn:
                    return s_xsw[g][:, l - g0, :, :]
            raise IndexError(l)

        s0mat = fin.tile([64, 512], F32, tag="s0mat")
        s1mat = fin.tile([64, 512], F32, tag="s1mat")

        # ---- main loop ----
        from collections import deque

        pending = deque()

        def emit_stats(ps, taq, et, tau, t0, is_block_end):
            for rp in range(2):
                for j in range(4):
                    if j == 0:
                        rhs = taq[0:100, 2 * rp, :]
                        lhsT = s_rm[0:100, 0, :]
                    else:
                        rhs = et[0:94, rp, j - 1, :]
                        lhsT = s_rm[0:94, j, :]
                    nc.tensor.matmul(
                        ps[32 * tau : 32 * tau + 4, rp, :],
                        lhsT,
                        rhs,
                        start=(j == 0),
                        stop=(j == 3),
                        skip_group_check=True,
                        tile_position=(0, 32 * tau),
                    )
            if is_block_end:
                sd = dpool.tile([128, 2, 512], F32, tag="sd")
                nc.vector.tensor_copy(sd[0:100, :, :], ps[0:100, :, :])
                for t2 in range(4):
                    j0 = 4 * (t0 + t2)
                    for rp in range(2):
                        eng = nc.sync if rp == 0 else nc.gpsimd
                        eng.dma_start(
                            out=s0mat[j0 + 2 * rp : j0 + 2 * rp + 2, :],
                            in_=sd[32 * t2 : 32 * t2 + 2, rp, :],
                        )
                        eng.dma_start(
                            out=s1mat[j0 + 2 * rp : j0 + 2 * rp + 2, :],
                            in_=sd[32 * t2 + 2 : 32 * t2 + 4, rp, :],
                        )

        ps = None
        for t in range(16):
            if t % 4 == 0:
                ps = pstat.tile([128, 2, 512], F32, tag="ps")
            tau = t % 4
            pw = paw.tile([128, 4, 512], F32, tag="pw")
            taq = epool.tile([128, 4, 512], BF16, tag="taq")
            for rp in range(2):
                rhs = xsw_row(t + rp).rearrange("p q s -> p (q s)")
                nc.tensor.matmul(
                    pw[0:100, 2 * rp, :], s_aw[(rp, 0)], rhs, start=True, stop=True
                )
                nc.tensor.matmul(
                    pw[0:94, 2 * rp + 1, :], s_aw[(rp, 1)], rhs, start=True, stop=True
                )
            nc.scalar.activation(
                taq[0:100, :, :], pw[0:100, :, :], exp_fn, scale=-1.0
            )
            # reconstruction: E_j = E_{j-1} * q  (bf16 2x, both rp per op)
            et = erp.tile([128, 2, 3, 512], BF16, tag="et")
            taq_v = taq.rearrange("p (rp aw) s -> p rp aw s", rp=2)
            nc.vector.tensor_mul(
                et[0:94, :, 0, :], taq_v[0:94, :, 0, :], taq_v[0:94, :, 1, :]
            )
            for j in (1, 2):
                nc.vector.tensor_mul(
                    et[0:94, :, j, :], et[0:94, :, j - 1, :], taq_v[0:94, :, 1, :]
                )
            pending.append((ps, taq, et, tau, t - tau, tau == 3))
            if len(pending) > 1:
                emit_stats(*pending.popleft())
        while pending:
            emit_stats(*pending.popleft())

        # ---- finalize: disp = S1 * recip(S0); un-permute (rw, s) -> w' ----
        rec = fin.tile([64, 512], F32, tag="rec")
        om = fin.tile([64, 512], F32, tag="om")
        nc.vector.reciprocal(rec, s0mat)
        nc.vector.tensor_mul(
            om.rearrange("j (s q) -> j q s", q=4),
            s1mat.rearrange("j (q s) -> j q s", q=4),
            rec.rearrange("j (q s) -> j q s", q=4),
        )
        nc.sync.dma_start(out=outp[:, :], in_=om)

    nc.compile()
    return nc


_CACHE: dict = {}


def _shard_inputs(x: np.ndarray):
    """Edge-pad and slice per-core shards (layout + dtype cast only)."""
    xpad = np.pad(x[:, 0], ((0, 0), (1, 1), (1, 3), (1, 1)), mode="edge")
    aw, rm = _build_consts()
    bf = mybir.dt.np(BF16)
    in_maps = []
    for c in range(NCORES):
        b, q = divmod(c, 4)
        xs = xpad[b][:, 16 * q : 16 * q + 18, :]          # [50, 18, 130]
        xsd = np.concatenate([xs[:, 0:17, :], xs[:, 1:18, :]], axis=0)
        xsdA = np.ascontiguousarray(xsd.reshape(100, NROW * 130)).astype(bf)
        xsdB = np.zeros_like(xsd)
        xsdB[:, :, 0:129] = xsd[:, :, 1:130]
        xsdB = np.ascontiguousarray(xsdB.reshape(100, NROW * 130)).astype(bf)
        in_maps.append({"xsdA": xsdA, "xsdB": xsdB, "aw": aw, "rm": rm})
    return in_maps


def kernel(x: np.ndarray, _trace: bool = False, _tmpdir=None):
    x = np.asarray(x, dtype=np.float32)
    assert x.shape == (2, 1, 48, 64, 128), x.shape
    if "nc" not in _CACHE:
        _CACHE["nc"] = _build_nc()
    nc = _CACHE["nc"]
    in_maps = _shard_inputs(x)
    res = run_bass_kernel_spmd(
        nc, in_maps, list(range(NCORES)), trace=_trace, tmpdir=_tmpdir
    )
    out = np.zeros((2, 256, 512), dtype=np.float32)
    for c in range(NCORES):
        b, q = divmod(c, 4)
        out[b, 64 * q : 64 * (q + 1), :] = res.results[c]["out"]
    if _trace:
        return out, res
    return out


# revision 14
# speedup vs baseline: 1.4240x; 1.0104x over previous
"""Trainium2 Bass kernel for nn_Disp_61125974557155 (V2: segment-geometric exp).

Computes: trilinear upsample of a cost volume [B,1,48,64,128] ->
[B,193,256,512] (align_corners=False), softmin over disparity,
disparity regression -> [B,256,512].

Key idea: along d', the upsampled volume is piecewise-linear over 49
low-res segments (~4 d' each), so exp(-vol) is piecewise GEOMETRIC:
within segment s, e_j = A_s * q_s^j with A_s = exp(-(u_s + f0*Delta)),
q_s = exp(-c*Delta), c = 48/193. Instead of 193 exp rows per pixel the
scalar engine computes only ~194 rows per (t, r-pair) (A and q for two
H-phases), and the idle vector engine reconstructs the remaining rows
with chained bf16 multiplies (2x DVE mode). Edge segments (replicated
pads, Delta=0, 2 d's each) are folded into the j=0 stat weights; the
single 5-element segment gets its 5th row directly from the alpha
matmul (alpha + 4w column).

Per core (8 = 2 batches x 4 H'-quarters), t-major over 16 coarse rows:
  PE:  alpha/w matmul [100->100/94, 512] per (t, rp) (H-lerp folded),
       then 8 masked stat matmuls (M=4: S0/S1 x r-even/r-odd) per t,
       accumulated over layers j=0..3 in PSUM slots at partition 32*(t%4).
  ACT: one exp over [100, 4, 512] PSUM -> SBUF bf16 per t.
  DVE: W-lerp (bf16), 3 reconstruction TT muls per t, stat drains every
       4 t, partition-parallel finalize.
  DMA: stat scatter to pixel-major S0/S1, final divide, output.
"""

import numpy as np
from contextlib import ExitStack

import concourse.bass as bass
import concourse.bacc as bacc
import concourse.tile as tile
from concourse import mybir
from concourse.bass_utils import run_bass_kernel_spmd

F32 = mybir.dt.float32
BF16 = mybir.dt.bfloat16

MAXDISP = 192
DP = MAXDISP + 1
KD = 48
KP = KD + 2
NCORES = 8
C2 = KD / DP
WH = (0.625, 0.875, 0.125, 0.375)
NROW = 17
ROW_GROUPS = ((0, 1), (1, 1), (2, 2), (4, 4), (8, 4), (12, 4), (16, 1))


def _segments():
    segs = {}
    for dd in range(DP):
        i = (dd + 0.5) * C2 - 0.5
        fl = int(np.floor(i))
        segs.setdefault(fl, []).append((dd, i - fl))
    int_fls = [fl for fl in sorted(segs) if 0 <= fl <= 46]
    edge_fls = [-1, 47]
    five = [fl for fl in int_fls if len(segs[fl]) == 5]
    assert len(int_fls) == 47 and len(five) == 1
    return segs, int_fls, edge_fls, five[0]


def _build_consts():
    segs, int_fls, edge_fls, five_fl = _segments()

    def hrow(r, k, wt):
        v = np.zeros(100)
        v[k] = (1 - WH[r]) * wt
        v[50 + k] = WH[r] * wt
        return v

    awA = np.zeros((2, 100, 100))
    awW = np.zeros((2, 100, 94))
    for rp in range(2):
        for ui, r in enumerate((2 * rp, 2 * rp + 1)):
            for s, fl in enumerate(int_fls):
                klo = fl + 1
                f0 = segs[fl][0][1]
                awA[rp][:, 47 * ui + s] = hrow(r, klo, 1 - f0) + hrow(r, klo + 1, f0)
                awW[rp][:, 47 * ui + s] = hrow(r, klo + 1, C2) + hrow(r, klo, -C2)
            for ei, fl in enumerate(edge_fls):
                klo = fl + 1
                f0 = segs[fl][0][1]
                awA[rp][:, 94 + 2 * ui + ei] = (
                    hrow(r, klo, 1 - f0) + hrow(r, klo + 1, f0)
                )
            klo = five_fl + 1
            f0 = segs[five_fl][0][1]
            a_col = hrow(r, klo, 1 - f0) + hrow(r, klo + 1, f0)
            w_col = hrow(r, klo + 1, C2) + hrow(r, klo, -C2)
            awA[rp][:, 98 + ui] = a_col + 4 * w_col

    # stat cols ordered (S0_r0, S0_r1, S1_r0, S1_r1) for contiguous scatters
    rmA = np.zeros((100, 4))
    rmE = np.zeros((3, 100, 4))
    for ui in range(2):
        for s, fl in enumerate(int_fls):
            ds = [t[0] for t in segs[fl]]
            rmA[47 * ui + s, ui] = 1
            rmA[47 * ui + s, 2 + ui] = ds[0]
            for j in (1, 2, 3):
                rmE[j - 1][47 * ui + s, ui] = 1
                rmE[j - 1][47 * ui + s, 2 + ui] = ds[j]
        for ei, fl in enumerate(edge_fls):
            ds = [t[0] for t in segs[fl]]
            rmA[94 + 2 * ui + ei, ui] = 2
            rmA[94 + 2 * ui + ei, 2 + ui] = ds[0] + ds[1]
        ds = [t[0] for t in segs[five_fl]]
        rmA[98 + ui, ui] = 1
        rmA[98 + ui, 2 + ui] = ds[4]

    bf = mybir.dt.np(BF16)
    aw = np.concatenate(
        [awA[0], awW[0], awA[1], awW[1]], axis=1
    )  # [100, 388]
    rm = np.concatenate([rmA] + [rmE[j] for j in range(3)], axis=1)  # [100, 16]
    return aw.astype(bf), rm.astype(bf)


def _build_nc() -> bass.Bass:
    nc = bacc.Bacc()
    # xsdA/xsdB: host-duplicated, B shifted by one w-col, so every lerp
    # slice starts at an even element offset (keeps DVE 2x eligibility).
    xsdA = nc.declare_dram_parameter("xsdA", [100, NROW * 130], BF16, isOutput=False)
    xsdB = nc.declare_dram_parameter("xsdB", [100, NROW * 130], BF16, isOutput=False)
    aw = nc.declare_dram_parameter("aw", [100, 388], BF16, isOutput=False)
    rm = nc.declare_dram_parameter("rm", [100, 16], BF16, isOutput=False)
    outp = nc.declare_dram_parameter("out", [64, 512], F32, isOutput=True)

    xa_v = xsdA.rearrange("p (h w) -> p h w", h=NROW)
    xb_v = xsdB.rearrange("p (h w) -> p h w", h=NROW)
    aw_v = aw  # cols: rp0A(100) rp0W(94) rp1A(100) rp1W(94)

    mult = mybir.AluOpType.mult
    add = mybir.AluOpType.add
    exp_fn = mybir.ActivationFunctionType.Exp

    with ExitStack() as ctx:
        tc = ctx.enter_context(tile.TileContext(nc))
        singles = ctx.enter_context(tc.tile_pool(name="singles", bufs=1))
        tmp_pool = ctx.enter_context(tc.tile_pool(name="tmp", bufs=2))
        epool = ctx.enter_context(tc.tile_pool(name="epool", bufs=4))
        erp = ctx.enter_context(tc.tile_pool(name="erp", bufs=3))
        dpool = ctx.enter_context(tc.tile_pool(name="dpool", bufs=2))
        fin = ctx.enter_context(tc.tile_pool(name="fin", bufs=1))
        paw = ctx.enter_context(tc.tile_pool(name="paw", bufs=1, space="PSUM"))
        pstat = ctx.enter_context(tc.tile_pool(name="pstat", bufs=2, space="PSUM"))

        # ---- loads ----
        s_xa, s_xb = [], []
        for g, (g0, gn) in enumerate(ROW_GROUPS):
            t_xa = singles.tile([100, gn, 130], BF16, tag=f"xa{g}")
            nc.sync.dma_start(out=t_xa, in_=xa_v[:, g0 : g0 + gn, :])
            t_xb = singles.tile([100, gn, 130], BF16, tag=f"xb{g}")
            nc.scalar.dma_start(out=t_xb, in_=xb_v[:, g0 : g0 + gn, :])
            s_xa.append(t_xa)
            s_xb.append(t_xb)
        s_aw = {}
        off = 0
        for rp in range(2):
            t_a = singles.tile([100, 100], BF16, tag=f"awa{rp}")
            nc.gpsimd.dma_start(out=t_a, in_=aw_v[:, off : off + 100])
            off += 100
            t_w = singles.tile([100, 94], BF16, tag=f"aww{rp}")
            nc.gpsimd.dma_start(out=t_w, in_=aw_v[:, off : off + 94])
            off += 94
            s_aw[(rp, 0)] = t_a
            s_aw[(rp, 1)] = t_w
        s_rm = singles.tile([100, 4, 4], BF16, tag="rm")
        nc.gpsimd.dma_start(out=s_rm, in_=rm.rearrange("p (j c) -> p j c", j=4))

        # ---- W-lerp (bf16): rw0/1 = xsB[s] + c*(xsA[s]-xsB[s]);
        #      rw2/3 = xsA[s+2] + c*(xsB[s]-xsA[s+2])  (all even offsets) ----
        s_xsw = []
        for g, (g0, gn) in enumerate(ROW_GROUPS):
            t_w = singles.tile([100, gn, 4, 128], BF16, tag=f"xsw{g}")
            d0 = tmp_pool.tile([100, gn, 128], BF16, tag="d0")
            d1 = tmp_pool.tile([100, gn, 128], BF16, tag="d1")
            nc.vector.tensor_sub(d0, s_xa[g][:, :, 0:128], s_xb[g][:, :, 0:128])
            nc.vector.tensor_sub(d1, s_xb[g][:, :, 0:128], s_xa[g][:, :, 2:130])
            for rw, (coef, dt_, base) in enumerate(
                (
                    (0.375, 0, 0),
                    (0.125, 0, 0),
                    (0.875, 1, 2),
                    (0.625, 1, 2),
                )
            ):
                src_d = d0 if dt_ == 0 else d1
                src_x = (
                    s_xb[g][:, :, 0:128] if dt_ == 0 else s_xa[g][:, :, 2:130]
                )
                nc.vector.scalar_tensor_tensor(
                    out=t_w[:, :, rw, :],
                    in0=src_d,
                    scalar=coef,
                    in1=src_x,
                    op0=mult,
                    op1=add,
                )
            s_xsw.append(t_w)

        def xsw_row(l: int) -> bass.AP:
            for g, (g0, gn) in enumerate(ROW_GROUPS):
                if g0 <= l < g0 + gn:
                    return s_xsw[g][:, l - g0, :, :]
            raise IndexError(l)

        s0mat = fin.tile([64, 512], F32, tag="s0mat")
        s1mat = fin.tile([64, 512], F32, tag="s1mat")

        # ---- main loop ----
        from collections import deque

        pending = deque()

        def emit_stats(ps, taq, et, tau, t0, is_block_end):
            for rp in range(2):
                for j in range(4):
                    if j == 0:
                        rhs = taq[0:100, 2 * rp, :]
                        lhsT = s_rm[0:100, 0, :]
                    else:
                        rhs = et[0:94, rp, j - 1, :]
                        lhsT = s_rm[0:94, j, :]
                    nc.tensor.matmul(
                        ps[32 * tau : 32 * tau + 4, rp, :],
                        lhsT,
                        rhs,
                        start=(j == 0),
                        stop=(j == 3),
                        skip_group_check=True,
                        tile_position=(0, 32 * tau),
                    )
            if is_block_end:
                sd = dpool.tile([128, 2, 512], F32, tag="sd")
                nc.vector.tensor_copy(sd[0:100, :, :], ps[0:100, :, :])
                for t2 in range(4):
                    j0 = 4 * (t0 + t2)
                    for rp in range(2):
                        eng = nc.sync if rp == 0 else nc.gpsimd
                        eng.dma_start(
                            out=s0mat[j0 + 2 * rp : j0 + 2 * rp + 2, :],
                            in_=sd[32 * t2 : 32 * t2 + 2, rp, :],
                        )
                        eng.dma_start(
                            out=s1mat[j0 + 2 * rp : j0 + 2 * rp + 2, :],
                            in_=sd[32 * t2 + 2 : 32 * t2 + 4, rp, :],
                        )

        ps = None
        for t in range(16):
            if t % 4 == 0:
                ps = pstat.tile([128, 2, 512], F32, tag="ps")
            tau = t % 4
            pw = paw.tile([128, 4, 512], F32, tag="pw")
            taq = epool.tile([128, 4, 512], BF16, tag="taq")
            for rp in range(2):
                rhs = xsw_row(t + rp).rearrange("p q s -> p (q s)")
                nc.tensor.matmul(
                    pw[0:100, 2 * rp, :], s_aw[(rp, 0)], rhs, start=True, stop=True
                )
                nc.tensor.matmul(
                    pw[0:94, 2 * rp + 1, :], s_aw[(rp, 1)], rhs, start=True, stop=True
                )
            nc.scalar.activation(
                taq[0:100, :, :], pw[0:100, :, :], exp_fn, scale=-1.0
            )
            # reconstruction: E_j = E_{j-1} * q  (bf16 2x, both rp per op)
            et = erp.tile([128, 2, 3, 512], BF16, tag="et")
            taq_v = taq.rearrange("p (rp aw) s -> p rp aw s", rp=2)
            nc.vector.tensor_mul(
                et[0:94, :, 0, :], taq_v[0:94, :, 0, :], taq_v[0:94, :, 1, :]
            )
            for j in (1, 2):
                nc.vector.tensor_mul(
                    et[0:94, :, j, :], et[0:94, :, j - 1, :], taq_v[0:94, :, 1, :]
                )
            pending.append((ps, taq, et, tau, t - tau, tau == 3))
            if len(pending) > 1:
                emit_stats(*pending.popleft())
        while pending:
            emit_stats(*pending.popleft())

        # ---- finalize: disp = S1 * recip(S0); un-permute (rw, s) -> w' ----
        rec = fin.tile([64, 512], F32, tag="rec")
        om = fin.tile([64, 512], F32, tag="om")
        nc.vector.reciprocal(rec, s0mat)
        nc.vector.tensor_mul(
            om.rearrange("j (s q) -> j q s", q=4),
            s1mat.rearrange("j (q s) -> j q s", q=4),
            rec.rearrange("j (q s) -> j q s", q=4),
        )
        nc.sync.dma_start(out=outp[:, :], in_=om)

    nc.compile()
    return nc


_CACHE: dict = {}


def _shard_inputs(x: np.ndarray):
    """Edge-pad and slice per-core shards (layout + dtype cast only)."""
    xpad = np.pad(x[:, 0], ((0, 0), (1, 1), (1, 3), (1, 1)), mode="edge")
    aw, rm = _build_consts()
    bf = mybir.dt.np(BF16)
    in_maps = []
    for c in range(NCORES):
        b, q = divmod(c, 4)
        xs = xpad[b][:, 16 * q : 16 * q + 18, :]          # [50, 18, 130]
        xsd = np.concatenate([xs[:, 0:17, :], xs[:, 1:18, :]], axis=0)
        xsdA = np.ascontiguousarray(xsd.reshape(100, NROW * 130)).astype(bf)
        xsdB = np.zeros_like(xsd)
        xsdB[:, :, 0:129] = xsd[:, :, 1:130]
        xsdB = np.ascontiguousarray(xsdB.reshape(100, NROW * 130)).astype(bf)
        in_maps.append({"xsdA": xsdA, "xsdB": xsdB, "aw": aw, "rm": rm})
    return in_maps


def kernel(x: np.ndarray, _trace: bool = False, _tmpdir=None):
    x = np.asarray(x, dtype=np.float32)
    assert x.shape == (2, 1, 48, 64, 128), x.shape
    if "nc" not in _CACHE:
        _CACHE["nc"] = _build_nc()
    nc = _CACHE["nc"]
    in_maps = _shard_inputs(x)
    res = run_bass_kernel_spmd(
        nc, in_maps, list(range(NCORES)), trace=_trace, tmpdir=_tmpdir
    )
    out = np.zeros((2, 256, 512), dtype=np.float32)
    for c in range(NCORES):
        b, q = divmod(c, 4)
        out[b, 64 * q : 64 * (q + 1), :] = res.results[c]["out"]
    if _trace:
        return out, res
    return out
